# revision 1
# baseline (speedup 1.0000x reference)
"""Fused masked-softmax attention (DotProductAttention) for 8 TRN2 NeuronCores.

Problem: B=16 batches of Q[2048,64] @ K[2048,64]^T -> mask cols >= valid_len
to -1e6 -> softmax -> @ V[2048,64].

Work decomposition: each batch splits into 4 q-quarters of 512 rows (one
PSUM-bank-wide q-tile each) -> 64 independent units.  Units are sorted by
valid k-tile count nv = ceil(valid_len/128) and dealt into 8 SPMD slots of
8 units (one per core); the compiled program runs slot s with a static
nv_s = max over that slot's units.  K-tiles wholly past a unit's valid_len
contribute exactly 0 (the mask row drives exp to underflow), so the extra
tiles cores run inside a slot are harmless and skipped tiles are exact.

Per-unit kernel (all on-chip, scores never touch HBM):
  * Layout: S^T[k, q] so softmax's k-reduction becomes a matmul and the
    attn @ V contraction needs no transpose of the big matrix.
  * mm1:  S^T chunk [128k, 512q] = kTa[:, ktile].T @ qTa with AUGMENTED
    bf16 operands: kTa = [K^T; mask_row] (65 rows), qTa = [Q^T; ones].
  * exp:  ACT engine, exp(0.125 * x) straight out of PSUM in merged
    N<=1536 activations, bf16 out into a PER-SLOT persistent SBUF tile.
  * mm2:  O^T_aug [65, 512q] = sum_k Vaug[ktile].T @ expS^T[ktile] with
    Vaug = [V | ones] (bf16) -> row 64 accumulates the softmax denominator
    in fp32 PSUM.
  * finish: copy PSUM->SBUF (f32), PE-transpose 128-col chunks back to
    [q, d] layout, reciprocal of denominator column, per-partition scale,
    one merged DMA per unit.

Scheduling (the v3 "deferred mm2" pipeline): TRN2's HAM duty-cycles the
PE: runs start at half clock (1.2GHz) and are promoted to 2.4GHz only
after a few 3.4us activity windows; promotion can arrive as late as
~25us on an unlucky core.  At half clock the PE cannot sustain
mm1+mm2 (2x512 cycles/k-tile) against the ACT engine's fixed exp pace
(512 cycles @1.2GHz/k-tile) -- the baseline lockstep mm1/exp/mm2 pipeline
therefore ran ACT-starved for the whole pre-promotion era.  Instead:
slots are processed big->small and each slot's mm2 burst is DEFERRED BY
TWO SLOTS (exps persist in per-slot SBUF tiles; the PSUM accumulator is
only occupied during the short burst).  Phases 0-1 are pure mm1: at any
clock the PE feeds ACT faster than ACT drains (512 vs 512+overhead
cycles), so the exp pipeline never stalls during the half-clock era (two
biggest slots = ~16us of runway).  Once promoted, the PE's per-group
slack (~0.9us) drains the deferred mm2 backlog well before ACT finishes.
Ending before ~50us also dodges HAM's global demotion at ~51.5us.
"""

import functools
from itertools import zip_longest

import numpy as np
import ml_dtypes

import concourse.bacc as bacc
import concourse.tile as tile
from concourse import mybir
from concourse import bass_utils
from concourse.masks import make_identity

B, LQ, LKV, D = 16, 2048, 2048, 64
N_CORES = 8
KT = 128            # k-tile (partition dim of S^T)
QT = 512            # q-rows per unit (= PSUM bank free dim)
NKT = LKV // KT     # 16
NSLOT = (B * LQ) // (N_CORES * QT)  # 8 units per core
GROUP = 3           # max k-tiles per PSUM tile / merged activation
LAG = 2             # slots of mm2 deferral (pure-mm1 runway for HAM ramp)
MASK_RAW = -8.0e6   # * 0.125 scale == -1e6 (reference MASK_VALUE)
F32 = mybir.dt.float32
BF16 = mybir.dt.bfloat16


def _widths(nv):
    """Split nv k-tiles into activation groups of width <=3, avoiding 1-wide
    groups (measured regression) where possible.  2-wide groups go FIRST so
    each slot's first activation has the shortest possible mm1 prefix."""
    threes, rem = divmod(nv, 3)
    if rem == 0:
        return [3] * threes
    if rem == 2:
        return [2] + [3] * threes
    if threes >= 1:
        return [2, 2] + [3] * (threes - 1)
    return [1]


@functools.lru_cache(maxsize=4)
def _build_module(nv_slots):
    nc = bacc.Bacc(None)
    qta_d = nc.dram_tensor("qta", [NSLOT, D + 1, QT], BF16, kind="ExternalInput")
    kta_d = nc.dram_tensor("kta", [NSLOT, D + 1, LKV], BF16, kind="ExternalInput")
    vau_d = nc.dram_tensor("vaug", [128, NSLOT * NKT * (D + 1)], BF16, kind="ExternalInput")
    out_d = nc.dram_tensor("o", [NSLOT, QT, D], F32, kind="ExternalOutput")

    slot_groups = []
    for nv in nv_slots:
        groups, g = [], 0
        for w in _widths(nv):
            groups.append((g, w))
            g += w
        assert g == nv
        slot_groups.append(groups)

    with tile.TileContext(nc) as tc:
        with (
            tc.tile_pool(name="weights", bufs=1) as wpool,
            tc.tile_pool(name="exps", bufs=1) as epool,
            tc.tile_pool(name="ot", bufs=2) as otpool,
            tc.tile_pool(name="recip", bufs=2) as rpool,
            tc.tile_pool(name="outs", bufs=2) as opool,
            tc.tile_pool(name="ps_s", bufs=2, space="PSUM") as ps_s,
            tc.tile_pool(name="ps_o", bufs=1, space="PSUM") as ps_o,
            tc.tile_pool(name="ps_t", bufs=1, space="PSUM") as ps_t,
        ):
            ident = wpool.tile([128, 128], F32, tag="ident")
            make_identity(nc, ident)

            # PE warm-up: fp32 matmuls on the identity bridging from the
            # preamble (~6.8us) to the first input DMA (~9.7us).  HAM grants
            # full clock only after a ~3.4us window of near-saturated
            # HIGH-INTENSITY PE work: fp32 128x128 matmuls qualify, while
            # cheap 65-row constant bf16 matmuls at 100% duty do NOT
            # (measured: every core then stayed at half clock until ~26us+).
            warm = ps_t.tile([128, 128], F32, tag="pt", name="warm")
            for _ in range(8):
                nc.tensor.matmul(warm, lhsT=ident, rhs=ident, start=True, stop=True)

            # Input loads (valid prefix only), spread across three HWDGE
            # rings so transfers and issue costs overlap: kta on the SP ring,
            # qta on the ACT ring, vaug on the DVE ring.  Slot order matches
            # consumption order (big -> small).
            kta_s = [
                wpool.tile(
                    [D + 1, nv_slots[s] * KT], BF16, tag=f"kta{s}", name=f"kta{s}"
                )
                for s in range(NSLOT)
            ]
            qta_s = [
                wpool.tile([D + 1, QT], BF16, tag=f"qta{s}", name=f"qta{s}")
                for s in range(NSLOT)
            ]
            vaug_s = [
                wpool.tile(
                    [128, nv_slots[s] * (D + 1)], BF16, tag=f"vaug{s}", name=f"vaug{s}"
                )
                for s in range(NSLOT)
            ]
            exps_s = [
                epool.tile(
                    [128, nv_slots[s] * QT], BF16, tag=f"exps{s}", name=f"exps{s}"
                )
                for s in range(NSLOT)
            ]

            # The ACT ring issues ONLY qta0 (its sequencer must be free for
            # the first ACTIVATE); kta/qta bulk goes on the SP ring in
            # consumption order; vaug (not needed until phase 2) on the
            # GpSimd ring so it doesn't contend with the critical head.
            c0 = slot_groups[0][0][1] * KT
            nc.sync.dma_start(out=kta_s[0][:, :c0], in_=kta_d[0, :, :c0])
            nc.scalar.dma_start(out=qta_s[0], in_=qta_d[0])
            nc.sync.dma_start(
                out=kta_s[0][:, c0:], in_=kta_d[0, :, c0 : nv_slots[0] * KT]
            )
            for s in range(1, NSLOT):
                nc.sync.dma_start(out=kta_s[s], in_=kta_d[s, :, : nv_slots[s] * KT])
                nc.sync.dma_start(out=qta_s[s], in_=qta_d[s])
            # vaug is first needed in phase 2 (~27us): the first half rides
            # the tail of the SP ring (after kta/qta, so it cannot contend
            # with the critical head transfers); the rest goes via the
            # GpSimd SWDGE ring.
            for s in range(NSLOT):
                ring = nc.sync if s < 4 else nc.gpsimd
                ring.dma_start(
                    out=vaug_s[s],
                    in_=vau_d[:, s * NKT * (D + 1) : (s * NKT + nv_slots[s]) * (D + 1)],
                )

            def finish(s, po):
                """Normalize po [65, 512] and store as out[s].  The last two
                slots' PSUM->SBUF copies run on the ACT engine (idle once the
                final exp is out) so the tail finish chains pipeline across
                ACT/PE/DVE instead of serializing on the DVE."""
                ot = otpool.tile([D + 1, QT], F32, tag="ot", name="ot")
                if s >= NSLOT - 2:
                    nc.scalar.copy(ot, po)
                else:
                    nc.vector.tensor_copy(ot, po)
                pt = ps_t.tile([128, QT // 128, D + 1], F32, tag="pt", name="pt")
                for j in range(QT // 128):
                    nc.tensor.transpose(
                        pt[:, j, :],
                        ot[:, j * 128 : (j + 1) * 128],
                        ident[: D + 1, : D + 1],
                    )
                rc = rpool.tile([128, QT // 128], F32, tag="rc", name="rc")
                nc.vector.reciprocal(rc, pt[:, :, D])
                ob = opool.tile([128, QT // 128, D], F32, tag="ob", name="ob")
                for j in range(QT // 128):
                    nc.vector.tensor_scalar_mul(
                        ob[:, j, :], pt[:, j, :D], rc[:, j : j + 1]
                    )
                out_ap = out_d[s].rearrange("(j p) d -> p j d", p=128)
                nc.sync.dma_start(out=out_ap, in_=ob)

            def emit_mm1(s, g, w):
                """Score group + merged exp for slot s, k-tiles [g, g+w)."""
                st = ps_s.tile([128, GROUP * QT], F32, tag="st", name="st")
                for j in range(w):
                    n = g + j
                    nc.tensor.matmul(
                        st[:, j * QT : (j + 1) * QT],
                        lhsT=kta_s[s][:, n * KT : (n + 1) * KT],
                        rhs=qta_s[s],
                        start=True,
                        stop=True,
                    )
                nc.scalar.activation(
                    out=exps_s[s][:, g * QT : (g + w) * QT],
                    in_=st[:, : w * QT],
                    func=mybir.ActivationFunctionType.Exp,
                    scale=0.125,
                )

            po_of = {}

            def emit_mm2(s, g, w):
                """Deferred attn@V accumulation for slot s, k-tiles [g, g+w)."""
                nv = nv_slots[s]
                if g == 0:
                    po_of[s] = ps_o.tile([D + 1, QT], F32, tag="po", name="po")
                po = po_of[s]
                for j in range(w):
                    n = g + j
                    nc.tensor.matmul(
                        po,
                        lhsT=vaug_s[s][:, n * (D + 1) : (n + 1) * (D + 1)],
                        rhs=exps_s[s][:, n * QT : (n + 1) * QT],
                        start=(n == 0),
                        stop=(n == nv - 1),
                        skip_group_check=True,
                    )
                if g + w == nv:
                    finish(s, po)

            # Slots 0-1 run the baseline lockstep (mm1 group -> exp -> mm2
            # group): real mm1+mm2 work keeps the PE saturated with
            # high-intensity matmuls through HAM's promotion windows.  From
            # slot 2 on, mm2 bursts are DEFERRED and drain from a
            # slot-ordered queue under a per-phase tile budget: light in
            # phases 2-3 (an unluckily-late promotion may still have the PE
            # at half clock, where mm2 drag starves the exp pipeline), heavy
            # once full clock is certain, so the backlog is gone by the time
            # the last exp lands.
            m2q = [(s, g, w) for s in range(LAG, NSLOT) for (g, w) in slot_groups[s]]
            qi = 0
            budget_tenths = [0, 0, 10, 14, 18, 20, 20, 20]
            for ph in range(NSLOT):
                m1 = slot_groups[ph]
                budget = (nv_slots[ph] * budget_tenths[ph] + 9) // 10
                for one in m1:
                    if ph >= LAG:
                        while budget > 0 and qi < len(m2q) and m2q[qi][0] <= ph - 1:
                            s2, g2, w2 = m2q[qi]
                            if w2 > budget:
                                break
                            emit_mm2(s2, g2, w2)
                            budget -= w2
                            qi += 1
                    emit_mm1(ph, *one)
                    if ph < LAG:
                        emit_mm2(ph, *one)
            while qi < len(m2q):
                emit_mm2(*m2q[qi])
                qi += 1

    nc.compile()
    return nc


def _plan(valid_lens):
    """Sort the 64 (batch, q-quarter) units by valid k-tile count and deal
    them into NSLOT slots of one unit per core.  Returns (core_units,
    nv_slots) where core_units[c][s] = (batch, quarter)."""
    VL = np.asarray(valid_lens).astype(np.int64)
    nv = np.maximum(1, np.minimum(NKT, (VL + KT - 1) // KT))
    qpb = LQ // QT  # quarters per batch
    unit_nv = np.repeat(nv, qpb)
    order = np.argsort(-unit_nv, kind="stable")
    core_units = [
        [(int(order[NSLOT * s + c]) // qpb, int(order[NSLOT * s + c]) % qpb) for s in range(NSLOT)]
        for c in range(N_CORES)
    ]
    nv_slots = tuple(int(unit_nv[order[NSLOT * s]]) for s in range(NSLOT))
    return core_units, nv_slots


def _shard_inputs(queries, keys, values, valid_lens, core_units):
    """Host-side layout per core: stacked per-unit augmented operands."""
    Q = np.asarray(queries, dtype=np.float32)
    K = np.asarray(keys, dtype=np.float32)
    V = np.asarray(values, dtype=np.float32)
    VL = np.asarray(valid_lens).astype(np.int64)

    cols = np.arange(LKV, dtype=np.int64)
    ones_row = np.ones((1, QT), np.float32)
    in_maps = []
    for c in range(N_CORES):
        qta = np.empty((NSLOT, D + 1, QT), np.float32)
        kta = np.empty((NSLOT, D + 1, LKV), np.float32)
        va = np.empty((128, NSLOT * NKT * (D + 1)), np.float32)
        for s, (b, qt) in enumerate(core_units[c]):
            qta[s] = np.concatenate(
                [Q[b, qt * QT : (qt + 1) * QT, :].T, ones_row], axis=0
            )
            mask = np.where(cols >= VL[b], MASK_RAW, 0.0).astype(np.float32)
            kta[s] = np.concatenate([K[b].T, mask[None, :]], axis=0)
            vb = np.concatenate([V[b], np.ones((LKV, 1), np.float32)], axis=-1)
            va[:, s * NKT * (D + 1) : (s + 1) * NKT * (D + 1)] = (
                vb.reshape(NKT, KT, D + 1).transpose(1, 0, 2).reshape(128, -1)
            )
        in_maps.append(
            {
                "qta": qta.astype(ml_dtypes.bfloat16),
                "kta": kta.astype(ml_dtypes.bfloat16),
                "vaug": va.astype(ml_dtypes.bfloat16),
            }
        )
    return in_maps


def kernel(queries, keys, values, valid_lens):
    core_units, nv_slots = _plan(valid_lens)
    nc = _build_module(nv_slots)
    in_maps = _shard_inputs(queries, keys, values, valid_lens, core_units)
    res = bass_utils.run_bass_kernel_spmd(nc, in_maps, core_ids=list(range(N_CORES)))
    out = np.empty((B, LQ, D), np.float32)
    for c in range(N_CORES):
        o = res.results[c]["o"].reshape(NSLOT, QT, D)
        for s, (b, qt) in enumerate(core_units[c]):
            out[b, qt * QT : (qt + 1) * QT, :] = o[s]
    return out



# revision 4
# speedup vs baseline: 1.0987x; 1.0987x over previous
"""Fused masked-softmax attention (DotProductAttention) for 8 TRN2 NeuronCores.

Problem: B=16 batches of Q[2048,64] @ K[2048,64]^T -> mask cols >= valid_len
to -1e6 -> softmax -> @ V[2048,64].

Work decomposition: each batch splits into 4 q-quarters of 512 rows -> 64
units.  Units are sorted by valid k-tile count nv = ceil(valid_len/128) and
dealt into 8 SPMD slots of 8 units (one per core); the compiled program
runs slot s with a static nv_s = max over that slot's units.  K-tiles
wholly past a unit's valid_len contribute exactly 0 (the mask row drives
exp to underflow), so extra tiles are harmless and skipped tiles exact.

v4 design ("ACT is the roofline"): the scalar engine's exp throughput
(1 elem/cycle/lane @1.2GHz = 427ns per 128x512 k-tile) is the hard floor
(~28.6us for the ~67 k-tiles/core this input needs).  Everything else is
arranged to keep ACT saturated from ~9us to the end:

  * mm1:  S^T chunk [128k, 512q] = kTa.T @ qTa with AUGMENTED bf16
    operands kTa=[K^T; mask_row], qTa=[Q^T; ones] (65-deep contraction).
  * exp:  ACT engine, exp(0.125*x), PSUM -> one big persistent SBUF tile
    (exps_all).  Score groups are GLOBAL (flat across slot boundaries):
    uniform 3-wide merged activations minimize the ~150ns/instr bubble.
  * mm2:  O^T_aug [65, 512q] = sum_k Vaug[kt].T @ expS^T[kt], Vaug=[V|ones]
    -> row 64 = softmax denominator in fp32 PSUM.
  * finish: DVE copies PSUM->SBUF [65,512], DMA straight out.  The
    division by the denominator AND the transpose back to [q, d] happen
    ON HOST (numpy) - no PE transposes, no reciprocal, no identity matrix,
    no gpsimd anywhere.

Scheduling: HAM duty-cycles the PE (cold 1.2GHz until ~3.4us of sustained
high-intensity work; bf16 65-row matmuls alone do NOT promote).  A short
fp32 warmup burst (DVE-memset tile) runs during the input DMAs, and the
first RUNWAY act-groups are pure mm1 (cold mm1 feeds ACT with margin
1335 < 1640 ns/group).  mm2 is deferred into a queue drained between
groups under a per-group budget: light while possibly cold, heavy once
warm.  Optional fp32 N=128 filler MMs keep PE duty high during the runway
so the MID window doesn't demote the clock before the drain phase.
"""

import functools

import numpy as np
import ml_dtypes

import concourse.bacc as bacc
import concourse.tile as tile
from concourse import mybir
from concourse import bass_utils

B, LQ, LKV, D = 16, 2048, 2048, 64
N_CORES = 8
KT = 128            # k-tile (partition dim of S^T)
QT = 512            # q-rows per unit (= PSUM bank free dim)
NKT = LKV // KT     # 16
NSLOT = (B * LQ) // (N_CORES * QT)  # 8 units per core
GROUP = 3           # k-tiles per PSUM score tile / merged activation
MASK_RAW = -8.0e6   # * 0.125 scale == -1e6 (reference MASK_VALUE)
F32 = mybir.dt.float32
BF16 = mybir.dt.bfloat16

# --- schedule knobs ---
WARM_MM = 6         # fp32 warmup matmuls before the first mm1
RUNWAY = 6          # leading act-groups with zero mm2 drain
COLD_BUDGET = 1     # mm2 MMs per group while possibly cold
COLD_GROUPS = 3     # how many groups after RUNWAY use COLD_BUDGET
WARM_BUDGET = 5     # mm2 MMs per group once warm
FILL_MM = 8         # fp32 N=128 filler MMs per runway group (HAM hold)
MARGIN = 2          # mm2 item needs exps emitted >= MARGIN groups back


def _widths(nv):
    """Split nv k-tiles into mm2 burst groups of width <=3."""
    threes, rem = divmod(nv, 3)
    out = [3] * threes
    if rem:
        out.append(rem)
    return out


@functools.lru_cache(maxsize=4)
def _build_module(nv_slots):
    nc = bacc.Bacc(None)
    # kq slab per slot: [qta (512 cols) | kta (nv*128 cols)] -> one DMA each
    kq_d = nc.dram_tensor("kq", [NSLOT, D + 1, QT + LKV], BF16, kind="ExternalInput")
    vau_d = nc.dram_tensor("vaug", [128, NSLOT * NKT * (D + 1)], BF16, kind="ExternalInput")
    out_d = nc.dram_tensor("o", [NSLOT, D + 1, QT], F32, kind="ExternalOutput")

    ntile = sum(nv_slots)
    # global flat tile list: (slot, n) in consumption order
    tiles = [(s, n) for s in range(NSLOT) for n in range(nv_slots[s])]
    base = [sum(nv_slots[:s]) for s in range(NSLOT)]
    # global act groups: first group narrow (earliest possible ACT start),
    # then uniform 3-wide
    gwidths = []
    rem = ntile
    first = 2 if ntile % 3 == 2 else (1 if ntile % 3 == 1 else 3)
    gwidths.append(first)
    rem -= first
    while rem:
        w = min(3, rem)
        gwidths.append(w)
        rem -= w
    ngrp = len(gwidths)

    with tile.TileContext(nc) as tc:
        with (
            tc.tile_pool(name="weights", bufs=1) as wpool,
            tc.tile_pool(name="exps", bufs=1) as epool,
            tc.tile_pool(name="ot", bufs=2) as otpool,
            tc.tile_pool(name="ps_s", bufs=2, space="PSUM") as ps_s,
            tc.tile_pool(name="ps_o", bufs=2, space="PSUM") as ps_o,
        ):
            # Warmup operand: DVE-memset fp32 ones (no DMA, no gpsimd).
            wrm = wpool.tile([128, 128], F32, tag="wrm")
            nc.vector.memset(wrm, 1.0)

            kq_s = [
                wpool.tile(
                    [D + 1, QT + nv_slots[s] * KT], BF16, tag=f"kq{s}", name=f"kq{s}"
                )
                for s in range(NSLOT)
            ]
            qta_s = [kq_s[s][:, :QT] for s in range(NSLOT)]

            def kta(s, n):
                return kq_s[s][:, QT + n * KT : QT + (n + 1) * KT]

            vaug_s = [
                wpool.tile([128, nv_slots[s] * (D + 1)], BF16, tag=f"vaug{s}", name=f"vaug{s}")
                for s in range(NSLOT)
            ]
            exps_all = epool.tile([128, ntile * QT], BF16, tag="exps", name="exps_all")

            # Input DMAs, all on the SP ring in consumption order.  The head
            # chunk (qta0 + first-group kta0) goes first; vaug loads are
            # interleaved after each later slot so they land well before the
            # mm2 drain phase reaches them.
            c0 = QT + gwidths[0] * KT
            nc.sync.dma_start(out=kq_s[0][:, :c0], in_=kq_d[0, :, :c0])
            nc.sync.dma_start(
                out=kq_s[0][:, c0:], in_=kq_d[0, :, c0 : QT + nv_slots[0] * KT]
            )
            for s in range(1, NSLOT):
                nc.sync.dma_start(
                    out=kq_s[s], in_=kq_d[s, :, : QT + nv_slots[s] * KT]
                )
                nc.sync.dma_start(
                    out=vaug_s[s - 1],
                    in_=vau_d[:, (s - 1) * NKT * (D + 1) : ((s - 1) * NKT + nv_slots[s - 1]) * (D + 1)],
                )
            nc.sync.dma_start(
                out=vaug_s[NSLOT - 1],
                in_=vau_d[:, (NSLOT - 1) * NKT * (D + 1) : ((NSLOT - 1) * NKT + nv_slots[NSLOT - 1]) * (D + 1)],
            )

            def emit_warm(n):
                for _ in range(n):
                    wp = ps_o.tile([128, 128], F32, tag="po", name="warm")
                    nc.tensor.matmul(wp, lhsT=wrm, rhs=wrm, start=True, stop=True)

            emit_warm(WARM_MM)

            po_of = {}

            def emit_mm2(s, g, w):
                """Deferred attn@V accumulation for slot s, k-tiles [g, g+w)."""
                nv = nv_slots[s]
                if g == 0:
                    po_of[s] = ps_o.tile([D + 1, QT], F32, tag="po", name="po")
                po = po_of[s]
                for j in range(w):
                    n = g + j
                    t = base[s] + n
                    nc.tensor.matmul(
                        po,
                        lhsT=vaug_s[s][:, n * (D + 1) : (n + 1) * (D + 1)],
                        rhs=exps_all[:, t * QT : (t + 1) * QT],
                        start=(n == 0),
                        stop=(n == nv - 1),
                        skip_group_check=True,
                    )
                if g + w == nv:
                    ot = otpool.tile([D + 1, QT], F32, tag="ot", name="ot")
                    nc.vector.tensor_copy(ot, po)
                    nc.sync.dma_start(out=out_d[s], in_=ot)

            # mm2 queue: per-slot bursts in <=3-wide chunks, slot order.
            m2q = []
            for s in range(NSLOT):
                g = 0
                for w in _widths(nv_slots[s]):
                    m2q.append((s, g, w))
                    g += w
            qi = 0

            # act coverage (in tiles) after each emitted group
            cover = []
            acc = 0
            for w in gwidths:
                acc += w
                cover.append(acc)

            t0 = 0
            for gi in range(ngrp):
                w = gwidths[gi]
                # drain deferred mm2 under budget; exps must be MARGIN
                # groups back so the FIFO tensor queue never head-blocks
                if gi < RUNWAY:
                    budget = 0
                elif gi < RUNWAY + COLD_GROUPS:
                    budget = COLD_BUDGET
                else:
                    budget = WARM_BUDGET
                ready_cover = cover[gi - MARGIN] if gi >= MARGIN else 0
                while budget > 0 and qi < len(m2q):
                    s2, g2, w2 = m2q[qi]
                    if base[s2] + g2 + w2 > ready_cover or w2 > budget:
                        break
                    emit_mm2(s2, g2, w2)
                    budget -= w2
                    qi += 1
                # mm1 group gi -> one 3-bank PSUM tile -> merged exp
                st = ps_s.tile([128, GROUP * QT], F32, tag="st", name="st")
                for j in range(w):
                    s, n = tiles[t0 + j]
                    nc.tensor.matmul(
                        st[:, j * QT : (j + 1) * QT],
                        lhsT=kta(s, n),
                        rhs=qta_s[s],
                        start=True,
                        stop=True,
                    )
                if gi < RUNWAY and FILL_MM:
                    emit_warm(FILL_MM)
                nc.scalar.activation(
                    out=exps_all[:, t0 * QT : (t0 + w) * QT],
                    in_=st[:, : w * QT],
                    func=mybir.ActivationFunctionType.Exp,
                    scale=0.125,
                )
                t0 += w
            while qi < len(m2q):
                emit_mm2(*m2q[qi])
                qi += 1

    nc.compile()
    return nc


def _plan(valid_lens):
    """Sort the 64 (batch, q-quarter) units by valid k-tile count and deal
    them into NSLOT slots of one unit per core.  Returns (core_units,
    nv_slots) where core_units[c][s] = (batch, quarter)."""
    VL = np.asarray(valid_lens).astype(np.int64)
    nv = np.maximum(1, np.minimum(NKT, (VL + KT - 1) // KT))
    qpb = LQ // QT  # quarters per batch
    unit_nv = np.repeat(nv, qpb)
    order = np.argsort(-unit_nv, kind="stable")
    core_units = [
        [(int(order[NSLOT * s + c]) // qpb, int(order[NSLOT * s + c]) % qpb) for s in range(NSLOT)]
        for c in range(N_CORES)
    ]
    nv_slots = tuple(int(unit_nv[order[NSLOT * s]]) for s in range(NSLOT))
    return core_units, nv_slots


def _shard_inputs(queries, keys, values, valid_lens, core_units):
    """Host-side layout per core: stacked per-unit augmented operands."""
    Q = np.asarray(queries, dtype=np.float32)
    K = np.asarray(keys, dtype=np.float32)
    V = np.asarray(values, dtype=np.float32)
    VL = np.asarray(valid_lens).astype(np.int64)

    cols = np.arange(LKV, dtype=np.int64)
    ones_row = np.ones((1, QT), np.float32)
    in_maps = []
    for c in range(N_CORES):
        kq = np.zeros((NSLOT, D + 1, QT + LKV), np.float32)
        va = np.empty((128, NSLOT * NKT * (D + 1)), np.float32)
        for s, (b, qt) in enumerate(core_units[c]):
            kq[s, :, :QT] = np.concatenate(
                [Q[b, qt * QT : (qt + 1) * QT, :].T, ones_row], axis=0
            )
            mask = np.where(cols >= VL[b], MASK_RAW, 0.0).astype(np.float32)
            kq[s, :, QT : QT + LKV] = np.concatenate([K[b].T, mask[None, :]], axis=0)
            vb = np.concatenate([V[b], np.ones((LKV, 1), np.float32)], axis=-1)
            va[:, s * NKT * (D + 1) : (s + 1) * NKT * (D + 1)] = (
                vb.reshape(NKT, KT, D + 1).transpose(1, 0, 2).reshape(128, -1)
            )
        in_maps.append(
            {
                "kq": kq.astype(ml_dtypes.bfloat16),
                "vaug": va.astype(ml_dtypes.bfloat16),
            }
        )
    return in_maps


def _unshard(res, core_units):
    """Host finish: normalize by the denominator row and transpose."""
    out = np.empty((B, LQ, D), np.float32)
    for c in range(N_CORES):
        o = res.results[c]["o"]  # [NSLOT, 65, 512]
        for s, (b, qt) in enumerate(core_units[c]):
            ot = o[s]
            out[b, qt * QT : (qt + 1) * QT, :] = (ot[:D] / ot[D : D + 1]).T
    return out


def kernel(queries, keys, values, valid_lens):
    core_units, nv_slots = _plan(valid_lens)
    nc = _build_module(nv_slots)
    in_maps = _shard_inputs(queries, keys, values, valid_lens, core_units)
    res = bass_utils.run_bass_kernel_spmd(nc, in_maps, core_ids=list(range(N_CORES)))
    return _unshard(res, core_units)


# revision 5
# speedup vs baseline: 1.1576x; 1.0536x over previous
"""Fused masked-softmax attention (DotProductAttention) for 8 TRN2 NeuronCores.

Problem: B=16 batches of Q[2048,64] @ K[2048,64]^T -> mask cols >= valid_len
to -1e6 -> softmax -> @ V[2048,64].

Work decomposition: each batch splits into 4 q-quarters of 512 rows -> 64
units.  Units are sorted by valid k-tile count nv = ceil(valid_len/128) and
dealt into 8 SPMD slots of 8 units (one per core); the compiled program
runs slot s with a static nv_s = max over that slot's units.  K-tiles
wholly past a unit's valid_len contribute exactly 0 (the mask row drives
exp to underflow), so extra tiles are harmless and skipped tiles exact.

v4 design ("ACT is the roofline"): the scalar engine's exp throughput
(1 elem/cycle/lane @1.2GHz = 427ns per 128x512 k-tile) is the hard floor
(~28.6us for the ~67 k-tiles/core this input needs).  Everything else is
arranged to keep ACT saturated from ~9us to the end:

  * mm1:  S^T chunk [128k, 512q] = kTa.T @ qTa with AUGMENTED bf16
    operands kTa=[K^T; mask_row], qTa=[Q^T; ones] (65-deep contraction).
  * exp:  ACT engine, exp(0.125*x), PSUM -> one big persistent SBUF tile
    (exps_all).  Score groups are GLOBAL (flat across slot boundaries):
    uniform 3-wide merged activations minimize the ~150ns/instr bubble.
  * mm2:  O^T_aug [65, 512q] = sum_k Vaug[kt].T @ expS^T[kt], Vaug=[V|ones]
    -> row 64 = softmax denominator in fp32 PSUM.
  * finish: DVE copies PSUM->SBUF [65,512], DMA straight out.  The
    division by the denominator AND the transpose back to [q, d] happen
    ON HOST (numpy) - no PE transposes, no reciprocal, no identity matrix,
    no gpsimd anywhere.

Scheduling: HAM duty-cycles the PE (cold 1.2GHz until ~3.4us of sustained
high-intensity work; bf16 65-row matmuls alone do NOT promote).  A short
fp32 warmup burst (DVE-memset tile) runs during the input DMAs, and the
first RUNWAY act-groups are pure mm1 (cold mm1 feeds ACT with margin
1335 < 1640 ns/group).  mm2 is deferred into a queue drained between
groups under a per-group budget: light while possibly cold, heavy once
warm.  Optional fp32 N=128 filler MMs keep PE duty high during the runway
so the MID window doesn't demote the clock before the drain phase.
"""

import functools

import numpy as np
import ml_dtypes

import concourse.bacc as bacc
import concourse.tile as tile
from concourse import mybir
from concourse import bass_utils

B, LQ, LKV, D = 16, 2048, 2048, 64
N_CORES = 8
KT = 128            # k-tile (partition dim of S^T)
QT = 512            # q-rows per unit (= PSUM bank free dim)
NKT = LKV // KT     # 16
NSLOT = (B * LQ) // (N_CORES * QT)  # 8 units per core
GROUP = 3           # k-tiles per PSUM score tile / merged activation
MASK_RAW = -8.0e6   # * 0.125 scale == -1e6 (reference MASK_VALUE)
F32 = mybir.dt.float32
BF16 = mybir.dt.bfloat16

# --- schedule knobs ---
WARM_MM = 4         # fp32 warmup matmuls before the first mm1
RUNWAY = 3          # leading act-groups with zero mm2 drain
COLD_BUDGET = 1     # mm2 MMs per group while possibly cold
COLD_GROUPS = 2     # how many groups after RUNWAY use COLD_BUDGET
WARM_BUDGET = 5     # mm2 MMs per group once warm
FILL_MM = 2         # fp32 filler MMs per runway group>=2 (HAM duty hold);
                    # emitted AFTER the act so its count-semaphore wait
                    # does not include them
MARGIN = 2          # mm2 item needs exps emitted >= MARGIN groups back


def _widths(nv):
    """Split nv k-tiles into mm2 burst groups of width <=3."""
    threes, rem = divmod(nv, 3)
    out = [3] * threes
    if rem:
        out.append(rem)
    return out


@functools.lru_cache(maxsize=4)
def _build_module(nv_slots):
    nc = bacc.Bacc(None)
    # kq slab per slot: [qta (512 cols) | kta (nv*128 cols)] -> one DMA each
    kq_d = nc.dram_tensor("kq", [NSLOT, D + 1, QT + LKV], BF16, kind="ExternalInput")
    vau_d = nc.dram_tensor("vaug", [128, NSLOT * NKT * (D + 1)], BF16, kind="ExternalInput")
    out_d = nc.dram_tensor("o", [NSLOT, D + 1, QT], F32, kind="ExternalOutput")

    ntile = sum(nv_slots)
    # global flat tile list: (slot, n) in consumption order
    tiles = [(s, n) for s in range(NSLOT) for n in range(nv_slots[s])]
    base = [sum(nv_slots[:s]) for s in range(NSLOT)]
    # global act groups: first group narrow (earliest possible ACT start),
    # then uniform 3-wide
    gwidths = []
    rem = ntile
    first = 2 if ntile % 3 == 2 else (1 if ntile % 3 == 1 else 3)
    gwidths.append(first)
    rem -= first
    while rem:
        w = min(3, rem)
        gwidths.append(w)
        rem -= w
    ngrp = len(gwidths)

    with tile.TileContext(nc) as tc:
        with (
            tc.tile_pool(name="weights", bufs=1) as wpool,
            tc.tile_pool(name="exps", bufs=1) as epool,
            tc.tile_pool(name="ot", bufs=2) as otpool,
            tc.tile_pool(name="ps_s", bufs=2, space="PSUM") as ps_s,
            tc.tile_pool(name="ps_o", bufs=2, space="PSUM") as ps_o,
        ):
            # Warmup operand: DVE-memset fp32 ones (no DMA, no gpsimd).
            wrm = wpool.tile([128, 128], F32, tag="wrm")
            nc.vector.memset(wrm, 1.0)

            kq_s = [
                wpool.tile(
                    [D + 1, QT + nv_slots[s] * KT], BF16, tag=f"kq{s}", name=f"kq{s}"
                )
                for s in range(NSLOT)
            ]
            qta_s = [kq_s[s][:, :QT] for s in range(NSLOT)]

            def kta(s, n):
                return kq_s[s][:, QT + n * KT : QT + (n + 1) * KT]

            vaug_s = [
                wpool.tile([128, nv_slots[s] * (D + 1)], BF16, tag=f"vaug{s}", name=f"vaug{s}")
                for s in range(NSLOT)
            ]
            exps_all = epool.tile([128, ntile * QT], BF16, tag="exps", name="exps_all")

            # Input DMAs, all on the SP ring in consumption order.  The head
            # chunk (qta0 + first-group kta0) goes first; vaug loads are
            # interleaved after each later slot so they land well before the
            # mm2 drain phase reaches them.
            c0 = QT + gwidths[0] * KT
            nc.sync.dma_start(out=kq_s[0][:, :c0], in_=kq_d[0, :, :c0])
            nc.sync.dma_start(
                out=kq_s[0][:, c0:], in_=kq_d[0, :, c0 : QT + nv_slots[0] * KT]
            )
            for s in range(1, NSLOT):
                nc.sync.dma_start(
                    out=kq_s[s], in_=kq_d[s, :, : QT + nv_slots[s] * KT]
                )
                nc.sync.dma_start(
                    out=vaug_s[s - 1],
                    in_=vau_d[:, (s - 1) * NKT * (D + 1) : ((s - 1) * NKT + nv_slots[s - 1]) * (D + 1)],
                )
            nc.sync.dma_start(
                out=vaug_s[NSLOT - 1],
                in_=vau_d[:, (NSLOT - 1) * NKT * (D + 1) : ((NSLOT - 1) * NKT + nv_slots[NSLOT - 1]) * (D + 1)],
            )

            def emit_warm(n):
                for _ in range(n):
                    wp = ps_o.tile([128, 128], F32, tag="po", name="warm")
                    nc.tensor.matmul(wp, lhsT=wrm, rhs=wrm, start=True, stop=True)

            emit_warm(WARM_MM)

            po_of = {}

            def emit_mm2(s, g, w):
                """Deferred attn@V accumulation for slot s, k-tiles [g, g+w)."""
                nv = nv_slots[s]
                if g == 0:
                    po_of[s] = ps_o.tile([D + 1, QT], F32, tag="po", name="po")
                po = po_of[s]
                for j in range(w):
                    n = g + j
                    t = base[s] + n
                    nc.tensor.matmul(
                        po,
                        lhsT=vaug_s[s][:, n * (D + 1) : (n + 1) * (D + 1)],
                        rhs=exps_all[:, t * QT : (t + 1) * QT],
                        start=(n == 0),
                        stop=(n == nv - 1),
                        skip_group_check=True,
                    )
                if g + w == nv:
                    ot = otpool.tile([D + 1, QT], F32, tag="ot", name="ot")
                    # the last two slots finish after the final exp: their
                    # copies run on the (now idle) ACT engine so the tail
                    # chain does not serialize on the DVE
                    if s >= NSLOT - 2:
                        nc.scalar.copy(ot, po)
                    else:
                        nc.vector.tensor_copy(ot, po)
                    nc.sync.dma_start(out=out_d[s], in_=ot)

            # mm2 queue: per-slot bursts in <=3-wide chunks, slot order.
            m2q = []
            for s in range(NSLOT):
                g = 0
                for w in _widths(nv_slots[s]):
                    m2q.append((s, g, w))
                    g += w
            qi = 0

            # act coverage (in tiles) after each emitted group
            cover = []
            acc = 0
            for w in gwidths:
                acc += w
                cover.append(acc)

            t0 = 0
            for gi in range(ngrp):
                w = gwidths[gi]
                # drain deferred mm2 under budget; exps must be MARGIN
                # groups back so the FIFO tensor queue never head-blocks
                if gi < RUNWAY:
                    budget = 0
                elif gi < RUNWAY + COLD_GROUPS:
                    budget = COLD_BUDGET
                else:
                    budget = WARM_BUDGET
                ready_cover = cover[gi - MARGIN] if gi >= MARGIN else 0
                while budget > 0 and qi < len(m2q):
                    s2, g2, w2 = m2q[qi]
                    if base[s2] + g2 + w2 > ready_cover or w2 > budget:
                        break
                    emit_mm2(s2, g2, w2)
                    budget -= w2
                    qi += 1
                # mm1 group gi -> one 3-bank PSUM tile -> merged exp
                st = ps_s.tile([128, GROUP * QT], F32, tag="st", name="st")
                for j in range(w):
                    s, n = tiles[t0 + j]
                    nc.tensor.matmul(
                        st[:, j * QT : (j + 1) * QT],
                        lhsT=kta(s, n),
                        rhs=qta_s[s],
                        start=True,
                        stop=True,
                    )
                nc.scalar.activation(
                    out=exps_all[:, t0 * QT : (t0 + w) * QT],
                    in_=st[:, : w * QT],
                    func=mybir.ActivationFunctionType.Exp,
                    scale=0.125,
                )
                if 2 <= gi < RUNWAY + COLD_GROUPS and FILL_MM:
                    emit_warm(FILL_MM)
                t0 += w
            while qi < len(m2q):
                emit_mm2(*m2q[qi])
                qi += 1

    nc.compile()
    return nc


def _plan(valid_lens):
    """Sort the 64 (batch, q-quarter) units by valid k-tile count and deal
    them into NSLOT slots of one unit per core.  Returns (core_units,
    nv_slots) where core_units[c][s] = (batch, quarter)."""
    VL = np.asarray(valid_lens).astype(np.int64)
    nv = np.maximum(1, np.minimum(NKT, (VL + KT - 1) // KT))
    qpb = LQ // QT  # quarters per batch
    unit_nv = np.repeat(nv, qpb)
    order = np.argsort(-unit_nv, kind="stable")
    core_units = [
        [(int(order[NSLOT * s + c]) // qpb, int(order[NSLOT * s + c]) % qpb) for s in range(NSLOT)]
        for c in range(N_CORES)
    ]
    nv_slots = tuple(int(unit_nv[order[NSLOT * s]]) for s in range(NSLOT))
    return core_units, nv_slots


def _shard_inputs(queries, keys, values, valid_lens, core_units):
    """Host-side layout per core: stacked per-unit augmented operands."""
    Q = np.asarray(queries, dtype=np.float32)
    K = np.asarray(keys, dtype=np.float32)
    V = np.asarray(values, dtype=np.float32)
    VL = np.asarray(valid_lens).astype(np.int64)

    cols = np.arange(LKV, dtype=np.int64)
    ones_row = np.ones((1, QT), np.float32)
    in_maps = []
    for c in range(N_CORES):
        kq = np.zeros((NSLOT, D + 1, QT + LKV), np.float32)
        va = np.empty((128, NSLOT * NKT * (D + 1)), np.float32)
        for s, (b, qt) in enumerate(core_units[c]):
            kq[s, :, :QT] = np.concatenate(
                [Q[b, qt * QT : (qt + 1) * QT, :].T, ones_row], axis=0
            )
            mask = np.where(cols >= VL[b], MASK_RAW, 0.0).astype(np.float32)
            kq[s, :, QT : QT + LKV] = np.concatenate([K[b].T, mask[None, :]], axis=0)
            vb = np.concatenate([V[b], np.ones((LKV, 1), np.float32)], axis=-1)
            va[:, s * NKT * (D + 1) : (s + 1) * NKT * (D + 1)] = (
                vb.reshape(NKT, KT, D + 1).transpose(1, 0, 2).reshape(128, -1)
            )
        in_maps.append(
            {
                "kq": kq.astype(ml_dtypes.bfloat16),
                "vaug": va.astype(ml_dtypes.bfloat16),
            }
        )
    return in_maps


def _unshard(res, core_units):
    """Host finish: normalize by the denominator row and transpose."""
    out = np.empty((B, LQ, D), np.float32)
    for c in range(N_CORES):
        o = res.results[c]["o"]  # [NSLOT, 65, 512]
        for s, (b, qt) in enumerate(core_units[c]):
            ot = o[s]
            out[b, qt * QT : (qt + 1) * QT, :] = (ot[:D] / ot[D : D + 1]).T
    return out


def kernel(queries, keys, values, valid_lens):
    core_units, nv_slots = _plan(valid_lens)
    nc = _build_module(nv_slots)
    in_maps = _shard_inputs(queries, keys, values, valid_lens, core_units)
    res = bass_utils.run_bass_kernel_spmd(nc, in_maps, core_ids=list(range(N_CORES)))
    return _unshard(res, core_units)


# revision 6
# speedup vs baseline: 1.1636x; 1.0052x over previous
"""Fused masked-softmax attention (DotProductAttention) for 8 TRN2 NeuronCores.

Problem: B=16 batches of Q[2048,64] @ K[2048,64]^T -> mask cols >= valid_len
to -1e6 -> softmax -> @ V[2048,64].

Work decomposition: each batch splits into 4 q-quarters of 512 rows -> 64
units.  Units are sorted by valid k-tile count nv = ceil(valid_len/128) and
dealt into 8 SPMD slots of 8 units (one per core); the compiled program
runs slot s with a static nv_s = max over that slot's units.  K-tiles
wholly past a unit's valid_len contribute exactly 0 (the mask row drives
exp to underflow), so extra tiles are harmless and skipped tiles exact.

v4 design ("ACT is the roofline"): the scalar engine's exp throughput
(1 elem/cycle/lane @1.2GHz = 427ns per 128x512 k-tile) is the hard floor
(~28.6us for the ~67 k-tiles/core this input needs).  Everything else is
arranged to keep ACT saturated from ~9us to the end:

  * mm1:  S^T chunk [128k, 512q] = kTa.T @ qTa with AUGMENTED bf16
    operands kTa=[K^T; mask_row], qTa=[Q^T; ones] (65-deep contraction).
  * exp:  ACT engine, exp(0.125*x), PSUM -> one big persistent SBUF tile
    (exps_all).  Score groups are GLOBAL (flat across slot boundaries):
    uniform 3-wide merged activations minimize the ~150ns/instr bubble.
  * mm2:  O^T_aug [65, 512q] = sum_k Vaug[kt].T @ expS^T[kt], Vaug=[V|ones]
    -> row 64 = softmax denominator in fp32 PSUM.
  * finish: DVE copies PSUM->SBUF [65,512], DMA straight out.  The
    division by the denominator AND the transpose back to [q, d] happen
    ON HOST (numpy) - no PE transposes, no reciprocal, no identity matrix,
    no gpsimd anywhere.

Scheduling: HAM duty-cycles the PE (cold 1.2GHz until ~3.4us of sustained
high-intensity work; bf16 65-row matmuls alone do NOT promote).  A short
fp32 warmup burst (DVE-memset tile) runs during the input DMAs, and the
first RUNWAY act-groups are pure mm1 (cold mm1 feeds ACT with margin
1335 < 1640 ns/group).  mm2 is deferred into a queue drained between
groups under a per-group budget: light while possibly cold, heavy once
warm.  Optional fp32 N=128 filler MMs keep PE duty high during the runway
so the MID window doesn't demote the clock before the drain phase.
"""

import functools

import numpy as np
import ml_dtypes

import concourse.bacc as bacc
import concourse.tile as tile
from concourse import mybir
from concourse import bass_utils

B, LQ, LKV, D = 16, 2048, 2048, 64
N_CORES = 8
KT = 128            # k-tile (partition dim of S^T)
QT = 512            # q-rows per unit (= PSUM bank free dim)
NKT = LKV // KT     # 16
NSLOT = (B * LQ) // (N_CORES * QT)  # 8 units per core
GROUP = 3           # k-tiles per PSUM score tile / merged activation
MASK_RAW = -8.0e6   # * 0.125 scale == -1e6 (reference MASK_VALUE)
F32 = mybir.dt.float32
BF16 = mybir.dt.bfloat16

# --- schedule knobs ---
WARM_MM = 6         # fp32 warmup matmuls before the first mm1
DRAIN_START = 7     # first act-group index that drains deferred mm2
FILL_MM = 4         # fp32 filler MMs per pre-drain group (HAM duty hold);
                    # emitted AFTER the act so its count-semaphore wait
                    # does not include them
MARGIN = 2          # mm2 item needs exps emitted >= MARGIN groups back


def _widths(nv):
    """Split nv k-tiles into mm2 burst groups of width <=3."""
    threes, rem = divmod(nv, 3)
    out = [3] * threes
    if rem:
        out.append(rem)
    return out


@functools.lru_cache(maxsize=4)
def _build_module(nv_slots):
    nc = bacc.Bacc(None)
    # kq slab per slot: [qta (512 cols) | kta (nv*128 cols)] -> one DMA each
    kq_d = nc.dram_tensor("kq", [NSLOT, D + 1, QT + LKV], BF16, kind="ExternalInput")
    vau_d = nc.dram_tensor("vaug", [128, NSLOT * NKT * (D + 1)], BF16, kind="ExternalInput")
    out_d = nc.dram_tensor("o", [NSLOT, D + 1, QT], F32, kind="ExternalOutput")

    ntile = sum(nv_slots)
    # global flat tile list: (slot, n) in consumption order
    tiles = [(s, n) for s in range(NSLOT) for n in range(nv_slots[s])]
    base = [sum(nv_slots[:s]) for s in range(NSLOT)]
    # global act groups: first group narrow (earliest possible ACT start),
    # then uniform 3-wide
    gwidths = []
    rem = ntile
    first = 2 if ntile % 3 == 2 else (1 if ntile % 3 == 1 else 3)
    gwidths.append(first)
    rem -= first
    while rem:
        w = min(3, rem)
        gwidths.append(w)
        rem -= w
    ngrp = len(gwidths)

    with tile.TileContext(nc) as tc:
        with (
            tc.tile_pool(name="weights", bufs=1) as wpool,
            tc.tile_pool(name="exps", bufs=1) as epool,
            tc.tile_pool(name="ot", bufs=2) as otpool,
            tc.tile_pool(name="ps_s", bufs=2, space="PSUM") as ps_s,
            tc.tile_pool(name="ps_o", bufs=2, space="PSUM") as ps_o,
        ):
            # Warmup operand: DVE-memset fp32 ones (no DMA, no gpsimd).
            wrm = wpool.tile([128, 128], F32, tag="wrm")
            nc.vector.memset(wrm, 1.0)

            kq_s = [
                wpool.tile(
                    [D + 1, QT + nv_slots[s] * KT], BF16, tag=f"kq{s}", name=f"kq{s}"
                )
                for s in range(NSLOT)
            ]
            qta_s = [kq_s[s][:, :QT] for s in range(NSLOT)]

            def kta(s, n):
                return kq_s[s][:, QT + n * KT : QT + (n + 1) * KT]

            vaug_s = [
                wpool.tile([128, nv_slots[s] * (D + 1)], BF16, tag=f"vaug{s}", name=f"vaug{s}")
                for s in range(NSLOT)
            ]
            exps_all = epool.tile([128, ntile * QT], BF16, tag="exps", name="exps_all")

            # Input DMAs, all on the SP ring in consumption order.  The head
            # chunk (qta0 + first-group kta0) goes first; vaug loads are
            # interleaved after each later slot so they land well before the
            # mm2 drain phase reaches them.
            # slot 0 in three chunks so early act-groups aren't gated on
            # one big transfer
            cuts = [0, QT + gwidths[0] * KT + GROUP * KT]
            cuts.append(min(QT + nv_slots[0] * KT, cuts[1] + 2 * GROUP * KT))
            cuts.append(QT + nv_slots[0] * KT)
            for a, b in zip(cuts, cuts[1:]):
                if b > a:
                    nc.sync.dma_start(out=kq_s[0][:, a:b], in_=kq_d[0, :, a:b])
            for s in range(1, NSLOT):
                nc.sync.dma_start(
                    out=kq_s[s], in_=kq_d[s, :, : QT + nv_slots[s] * KT]
                )
                nc.sync.dma_start(
                    out=vaug_s[s - 1],
                    in_=vau_d[:, (s - 1) * NKT * (D + 1) : ((s - 1) * NKT + nv_slots[s - 1]) * (D + 1)],
                )
            nc.sync.dma_start(
                out=vaug_s[NSLOT - 1],
                in_=vau_d[:, (NSLOT - 1) * NKT * (D + 1) : ((NSLOT - 1) * NKT + nv_slots[NSLOT - 1]) * (D + 1)],
            )

            def emit_warm(n):
                for _ in range(n):
                    wp = ps_o.tile([128, 128], F32, tag="po", name="warm")
                    nc.tensor.matmul(wp, lhsT=wrm, rhs=wrm, start=True, stop=True)

            emit_warm(WARM_MM)

            po_of = {}

            def emit_mm2(s, g, w):
                """Deferred attn@V accumulation for slot s, k-tiles [g, g+w)."""
                nv = nv_slots[s]
                if g == 0:
                    po_of[s] = ps_o.tile([D + 1, QT], F32, tag="po", name="po")
                po = po_of[s]
                for j in range(w):
                    n = g + j
                    t = base[s] + n
                    nc.tensor.matmul(
                        po,
                        lhsT=vaug_s[s][:, n * (D + 1) : (n + 1) * (D + 1)],
                        rhs=exps_all[:, t * QT : (t + 1) * QT],
                        start=(n == 0),
                        stop=(n == nv - 1),
                        skip_group_check=True,
                    )
                if g + w == nv:
                    ot = otpool.tile([D + 1, QT], F32, tag="ot", name="ot")
                    # the last two slots finish after the final exp: their
                    # copies run on the (now idle) ACT engine so the tail
                    # chain does not serialize on the DVE
                    if s == NSLOT - 1:
                        nc.scalar.copy(ot, po)
                    else:
                        nc.vector.tensor_copy(ot, po)
                    nc.sync.dma_start(out=out_d[s], in_=ot)

            # mm2 queue: per-slot bursts in <=3-wide chunks, slot order.
            m2q = []
            for s in range(NSLOT):
                g = 0
                for w in _widths(nv_slots[s]):
                    m2q.append((s, g, w))
                    g += w
            qi = 0

            # act coverage (in tiles) after each emitted group
            cover = []
            acc = 0
            for w in gwidths:
                acc += w
                cover.append(acc)

            t0 = 0
            for gi in range(ngrp):
                w = gwidths[gi]
                # drain deferred mm2 under budget; exps must be MARGIN
                # groups back so the FIFO tensor queue never head-blocks.
                # Sustainable drain rate is ~4.6 mm2/group (PE slack vs the
                # 1640ns act pace), so alternate 6/3 for an average of 4.5.
                if gi < DRAIN_START:
                    budget = 0
                else:
                    budget = 6 if (gi - DRAIN_START) % 2 == 0 else 3
                ready_cover = cover[gi - MARGIN] if gi >= MARGIN else 0
                while budget > 0 and qi < len(m2q):
                    s2, g2, w2 = m2q[qi]
                    if base[s2] + g2 + w2 > ready_cover or w2 > budget:
                        break
                    emit_mm2(s2, g2, w2)
                    budget -= w2
                    qi += 1
                # mm1 group gi -> one 3-bank PSUM tile -> merged exp
                st = ps_s.tile([128, GROUP * QT], F32, tag="st", name="st")
                for j in range(w):
                    s, n = tiles[t0 + j]
                    nc.tensor.matmul(
                        st[:, j * QT : (j + 1) * QT],
                        lhsT=kta(s, n),
                        rhs=qta_s[s],
                        start=True,
                        stop=True,
                    )
                nc.scalar.activation(
                    out=exps_all[:, t0 * QT : (t0 + w) * QT],
                    in_=st[:, : w * QT],
                    func=mybir.ActivationFunctionType.Exp,
                    scale=0.125,
                )
                if 1 <= gi < DRAIN_START and FILL_MM:
                    emit_warm(FILL_MM)
                t0 += w
            while qi < len(m2q):
                emit_mm2(*m2q[qi])
                qi += 1

    nc.compile()
    return nc


def _plan(valid_lens):
    """Sort the 64 (batch, q-quarter) units by valid k-tile count and deal
    them into NSLOT slots of one unit per core.  Returns (core_units,
    nv_slots) where core_units[c][s] = (batch, quarter)."""
    VL = np.asarray(valid_lens).astype(np.int64)
    nv = np.maximum(1, np.minimum(NKT, (VL + KT - 1) // KT))
    qpb = LQ // QT  # quarters per batch
    unit_nv = np.repeat(nv, qpb)
    order = np.argsort(-unit_nv, kind="stable")
    core_units = [
        [(int(order[NSLOT * s + c]) // qpb, int(order[NSLOT * s + c]) % qpb) for s in range(NSLOT)]
        for c in range(N_CORES)
    ]
    nv_slots = tuple(int(unit_nv[order[NSLOT * s]]) for s in range(NSLOT))
    return core_units, nv_slots


def _shard_inputs(queries, keys, values, valid_lens, core_units):
    """Host-side layout per core: stacked per-unit augmented operands."""
    Q = np.asarray(queries, dtype=np.float32)
    K = np.asarray(keys, dtype=np.float32)
    V = np.asarray(values, dtype=np.float32)
    VL = np.asarray(valid_lens).astype(np.int64)

    cols = np.arange(LKV, dtype=np.int64)
    ones_row = np.ones((1, QT), np.float32)
    in_maps = []
    for c in range(N_CORES):
        kq = np.zeros((NSLOT, D + 1, QT + LKV), np.float32)
        va = np.empty((128, NSLOT * NKT * (D + 1)), np.float32)
        for s, (b, qt) in enumerate(core_units[c]):
            kq[s, :, :QT] = np.concatenate(
                [Q[b, qt * QT : (qt + 1) * QT, :].T, ones_row], axis=0
            )
            mask = np.where(cols >= VL[b], MASK_RAW, 0.0).astype(np.float32)
            kq[s, :, QT : QT + LKV] = np.concatenate([K[b].T, mask[None, :]], axis=0)
            vb = np.concatenate([V[b], np.ones((LKV, 1), np.float32)], axis=-1)
            va[:, s * NKT * (D + 1) : (s + 1) * NKT * (D + 1)] = (
                vb.reshape(NKT, KT, D + 1).transpose(1, 0, 2).reshape(128, -1)
            )
        in_maps.append(
            {
                "kq": kq.astype(ml_dtypes.bfloat16),
                "vaug": va.astype(ml_dtypes.bfloat16),
            }
        )
    return in_maps


def _unshard(res, core_units):
    """Host finish: normalize by the denominator row and transpose."""
    out = np.empty((B, LQ, D), np.float32)
    for c in range(N_CORES):
        o = res.results[c]["o"]  # [NSLOT, 65, 512]
        for s, (b, qt) in enumerate(core_units[c]):
            ot = o[s]
            out[b, qt * QT : (qt + 1) * QT, :] = (ot[:D] / ot[D : D + 1]).T
    return out


def kernel(queries, keys, values, valid_lens):
    core_units, nv_slots = _plan(valid_lens)
    nc = _build_module(nv_slots)
    in_maps = _shard_inputs(queries, keys, values, valid_lens, core_units)
    res = bass_utils.run_bass_kernel_spmd(nc, in_maps, core_ids=list(range(N_CORES)))
    return _unshard(res, core_units)


# revision 7
# speedup vs baseline: 1.2169x; 1.0458x over previous
"""Fused masked-softmax attention (DotProductAttention) for 8 TRN2 NeuronCores.

Problem: B=16 batches of Q[2048,64] @ K[2048,64]^T -> mask cols >= valid_len
to -1e6 -> softmax -> @ V[2048,64].

Work decomposition: each batch splits into 4 q-quarters of 512 rows -> 64
units.  Units are sorted by valid k-tile count nv = ceil(valid_len/128) and
dealt into 8 SPMD slots of 8 units (one per core); the compiled program
runs slot s with a static nv_s = max over that slot's units.  K-tiles
wholly past a unit's valid_len contribute exactly 0 (the mask row drives
exp to underflow), so extra tiles are harmless and skipped tiles exact.

v4 design ("ACT is the roofline"): the scalar engine's exp throughput
(1 elem/cycle/lane @1.2GHz = 427ns per 128x512 k-tile) is the hard floor
(~28.6us for the ~67 k-tiles/core this input needs).  Everything else is
arranged to keep ACT saturated from ~9us to the end:

  * mm1:  S^T chunk [128k, 512q] = kTa.T @ qTa with AUGMENTED bf16
    operands kTa=[K^T; mask_row], qTa=[Q^T; ones] (65-deep contraction).
  * exp:  ACT engine, exp(0.125*x), PSUM -> one big persistent SBUF tile
    (exps_all).  Score groups are GLOBAL (flat across slot boundaries):
    uniform 3-wide merged activations minimize the ~150ns/instr bubble.
  * mm2:  O^T_aug [65, 512q] = sum_k Vaug[kt].T @ expS^T[kt], Vaug=[V|ones]
    -> row 64 = softmax denominator in fp32 PSUM.
  * finish: DVE copies PSUM->SBUF [65,512], DMA straight out.  The
    division by the denominator AND the transpose back to [q, d] happen
    ON HOST (numpy) - no PE transposes, no reciprocal, no identity matrix,
    no gpsimd anywhere.

Scheduling: HAM duty-cycles the PE (cold 1.2GHz until ~3.4us of sustained
high-intensity work; bf16 65-row matmuls alone do NOT promote).  A short
fp32 warmup burst (DVE-memset tile) runs during the input DMAs, and the
first RUNWAY act-groups are pure mm1 (cold mm1 feeds ACT with margin
1335 < 1640 ns/group).  mm2 is deferred into a queue drained between
groups under a per-group budget: light while possibly cold, heavy once
warm.  Optional fp32 N=128 filler MMs keep PE duty high during the runway
so the MID window doesn't demote the clock before the drain phase.
"""

import functools

import numpy as np
import ml_dtypes

import concourse.bacc as bacc
import concourse.tile as tile
from concourse import mybir
from concourse import bass_utils

B, LQ, LKV, D = 16, 2048, 2048, 64
N_CORES = 8
KT = 128            # k-tile (partition dim of S^T)
QT = 512            # q-rows per unit (= PSUM bank free dim)
NKT = LKV // KT     # 16
NSLOT = (B * LQ) // (N_CORES * QT)  # 8 units per core
GROUP = 3           # k-tiles per PSUM score tile / merged activation
MASK_RAW = -8.0e6   # * 0.125 scale == -1e6 (reference MASK_VALUE)
F32 = mybir.dt.float32
BF16 = mybir.dt.bfloat16

# --- schedule knobs ---
WARM_MM = 6         # fp32 warmup matmuls before the first mm1
DRAIN_START = 5     # first act-group index that drains deferred mm2
FILL_MM = 4         # fp32 filler MMs per pre-drain group (HAM duty hold);
                    # emitted AFTER the act so its count-semaphore wait
                    # does not include them
MARGIN = 2          # mm2 item needs exps emitted >= MARGIN groups back


def _widths(nv):
    """Split nv k-tiles into mm2 burst groups of width <=3."""
    threes, rem = divmod(nv, 3)
    out = [3] * threes
    if rem:
        out.append(rem)
    return out


@functools.lru_cache(maxsize=4)
def _build_module(nv_slots):
    nc = bacc.Bacc(None)
    # kq slab per slot: [qta (512 cols) | kta (nv*128 cols)] -> one DMA each
    kq_d = nc.dram_tensor("kq", [NSLOT, D + 1, QT + LKV], BF16, kind="ExternalInput")
    vau_d = nc.dram_tensor("vaug", [128, NSLOT * NKT * (D + 1)], BF16, kind="ExternalInput")
    out_d = nc.dram_tensor("o", [NSLOT, D + 1, QT], F32, kind="ExternalOutput")

    ntile = sum(nv_slots)
    # global flat tile list: (slot, n) in consumption order
    tiles = [(s, n) for s in range(NSLOT) for n in range(nv_slots[s])]
    base = [sum(nv_slots[:s]) for s in range(NSLOT)]
    # global act groups: first group narrow (earliest possible ACT start),
    # then uniform 3-wide
    gwidths = []
    rem = ntile
    first = 2 if ntile % 3 == 2 else (1 if ntile % 3 == 1 else 3)
    gwidths.append(first)
    rem -= first
    while rem:
        w = min(3, rem)
        gwidths.append(w)
        rem -= w
    ngrp = len(gwidths)

    with tile.TileContext(nc) as tc:
        with (
            tc.tile_pool(name="weights", bufs=1) as wpool,
            tc.tile_pool(name="exps", bufs=1) as epool,
            tc.tile_pool(name="ot", bufs=2) as otpool,
            tc.tile_pool(name="ps_s", bufs=2, space="PSUM") as ps_s,
            tc.tile_pool(name="ps_o", bufs=2, space="PSUM") as ps_o,
        ):
            # Warmup operand: DVE-memset fp32 ones (no DMA, no gpsimd).
            wrm = wpool.tile([128, 128], F32, tag="wrm")
            nc.vector.memset(wrm, 1.0)

            kq_s = [
                wpool.tile(
                    [D + 1, QT + nv_slots[s] * KT], BF16, tag=f"kq{s}", name=f"kq{s}"
                )
                for s in range(NSLOT)
            ]
            qta_s = [kq_s[s][:, :QT] for s in range(NSLOT)]

            def kta(s, n):
                return kq_s[s][:, QT + n * KT : QT + (n + 1) * KT]

            vaug_s = [
                wpool.tile([128, nv_slots[s] * (D + 1)], BF16, tag=f"vaug{s}", name=f"vaug{s}")
                for s in range(NSLOT)
            ]
            exps_all = epool.tile([128, ntile * QT], BF16, tag="exps", name="exps_all")

            # Input DMAs, all on the SP ring in consumption order.  The head
            # chunk (qta0 + first-group kta0) goes first; vaug loads are
            # interleaved after each later slot so they land well before the
            # mm2 drain phase reaches them.
            # slot 0 in three chunks so early act-groups aren't gated on
            # one big transfer
            cuts = [0, QT + gwidths[0] * KT + GROUP * KT]
            cuts.append(min(QT + nv_slots[0] * KT, cuts[1] + 2 * GROUP * KT))
            cuts.append(QT + nv_slots[0] * KT)
            for a, b in zip(cuts, cuts[1:]):
                if b > a:
                    nc.sync.dma_start(out=kq_s[0][:, a:b], in_=kq_d[0, :, a:b])
            for s in range(1, NSLOT):
                nc.sync.dma_start(
                    out=kq_s[s], in_=kq_d[s, :, : QT + nv_slots[s] * KT]
                )
                nc.sync.dma_start(
                    out=vaug_s[s - 1],
                    in_=vau_d[:, (s - 1) * NKT * (D + 1) : ((s - 1) * NKT + nv_slots[s - 1]) * (D + 1)],
                )
            nc.sync.dma_start(
                out=vaug_s[NSLOT - 1],
                in_=vau_d[:, (NSLOT - 1) * NKT * (D + 1) : ((NSLOT - 1) * NKT + nv_slots[NSLOT - 1]) * (D + 1)],
            )

            def emit_warm(n):
                for _ in range(n):
                    wp = ps_o.tile([128, 128], F32, tag="po", name="warm")
                    nc.tensor.matmul(wp, lhsT=wrm, rhs=wrm, start=True, stop=True)

            emit_warm(WARM_MM)

            po_of = {}

            def emit_mm2(s, g, w):
                """Deferred attn@V accumulation for slot s, k-tiles [g, g+w)."""
                nv = nv_slots[s]
                if g == 0:
                    po_of[s] = ps_o.tile([D + 1, QT], F32, tag="po", name="po")
                po = po_of[s]
                for j in range(w):
                    n = g + j
                    t = base[s] + n
                    nc.tensor.matmul(
                        po,
                        lhsT=vaug_s[s][:, n * (D + 1) : (n + 1) * (D + 1)],
                        rhs=exps_all[:, t * QT : (t + 1) * QT],
                        start=(n == 0),
                        stop=(n == nv - 1),
                        skip_group_check=True,
                    )
                if g + w == nv:
                    ot = otpool.tile([D + 1, QT], F32, tag="ot", name="ot")
                    if s == NSLOT - 1:
                        # tail-critical: split the finish across ACT+DVE and
                        # both DMA rings so the final chain is half as long
                        h = QT // 2
                        nc.scalar.copy(ot[:, :h], po[:, :h])
                        nc.scalar.dma_start(out=out_d[s, :, :h], in_=ot[:, :h])
                        nc.vector.tensor_copy(ot[:, h:], po[:, h:])
                        nc.sync.dma_start(out=out_d[s, :, h:], in_=ot[:, h:])
                    else:
                        nc.vector.tensor_copy(ot, po)
                        nc.sync.dma_start(out=out_d[s], in_=ot)

            # mm2 queue: per-slot bursts in <=3-wide chunks, slot order.
            m2q = []
            for s in range(NSLOT):
                g = 0
                for w in _widths(nv_slots[s]):
                    m2q.append((s, g, w))
                    g += w
            qi = 0

            # act coverage (in tiles) after each emitted group
            cover = []
            acc = 0
            for w in gwidths:
                acc += w
                cover.append(acc)

            t0 = 0
            for gi in range(ngrp):
                w = gwidths[gi]
                # drain deferred mm2 under budget; exps must be MARGIN
                # groups back so the FIFO tensor queue never head-blocks.
                # Sustainable drain rate is ~4.6 mm2/group (PE slack vs the
                # 1640ns act pace): light while possibly cold, then 5/4.
                if gi < DRAIN_START:
                    budget = 0
                elif gi < DRAIN_START + 2:
                    budget = 2
                else:
                    budget = 5 if (gi - DRAIN_START) % 2 == 0 else 4
                ready_cover = cover[gi - MARGIN] if gi >= MARGIN else 0
                while budget > 0 and qi < len(m2q):
                    s2, g2, w2 = m2q[qi]
                    if base[s2] + g2 + w2 > ready_cover or w2 > budget:
                        break
                    emit_mm2(s2, g2, w2)
                    budget -= w2
                    qi += 1
                # mm1 group gi -> one 3-bank PSUM tile -> merged exp
                st = ps_s.tile([128, GROUP * QT], F32, tag="st", name="st")
                for j in range(w):
                    s, n = tiles[t0 + j]
                    nc.tensor.matmul(
                        st[:, j * QT : (j + 1) * QT],
                        lhsT=kta(s, n),
                        rhs=qta_s[s],
                        start=True,
                        stop=True,
                    )
                nc.scalar.activation(
                    out=exps_all[:, t0 * QT : (t0 + w) * QT],
                    in_=st[:, : w * QT],
                    func=mybir.ActivationFunctionType.Exp,
                    scale=0.125,
                )
                if 1 <= gi < DRAIN_START and FILL_MM:
                    emit_warm(FILL_MM)
                
                t0 += w
            while qi < len(m2q):
                emit_mm2(*m2q[qi])
                qi += 1

    nc.compile()
    return nc


def _plan(valid_lens):
    """Sort the 64 (batch, q-quarter) units by valid k-tile count and deal
    them into NSLOT slots of one unit per core.  Returns (core_units,
    nv_slots) where core_units[c][s] = (batch, quarter)."""
    VL = np.asarray(valid_lens).astype(np.int64)
    nv = np.maximum(1, np.minimum(NKT, (VL + KT - 1) // KT))
    qpb = LQ // QT  # quarters per batch
    unit_nv = np.repeat(nv, qpb)
    order = np.argsort(-unit_nv, kind="stable")
    core_units = [
        [(int(order[NSLOT * s + c]) // qpb, int(order[NSLOT * s + c]) % qpb) for s in range(NSLOT)]
        for c in range(N_CORES)
    ]
    nv_slots = tuple(int(unit_nv[order[NSLOT * s]]) for s in range(NSLOT))
    return core_units, nv_slots


def _shard_inputs(queries, keys, values, valid_lens, core_units):
    """Host-side layout per core: stacked per-unit augmented operands."""
    Q = np.asarray(queries, dtype=np.float32)
    K = np.asarray(keys, dtype=np.float32)
    V = np.asarray(values, dtype=np.float32)
    VL = np.asarray(valid_lens).astype(np.int64)

    cols = np.arange(LKV, dtype=np.int64)
    ones_row = np.ones((1, QT), np.float32)
    in_maps = []
    for c in range(N_CORES):
        kq = np.zeros((NSLOT, D + 1, QT + LKV), np.float32)
        va = np.empty((128, NSLOT * NKT * (D + 1)), np.float32)
        for s, (b, qt) in enumerate(core_units[c]):
            kq[s, :, :QT] = np.concatenate(
                [Q[b, qt * QT : (qt + 1) * QT, :].T, ones_row], axis=0
            )
            mask = np.where(cols >= VL[b], MASK_RAW, 0.0).astype(np.float32)
            kq[s, :, QT : QT + LKV] = np.concatenate([K[b].T, mask[None, :]], axis=0)
            vb = np.concatenate([V[b], np.ones((LKV, 1), np.float32)], axis=-1)
            va[:, s * NKT * (D + 1) : (s + 1) * NKT * (D + 1)] = (
                vb.reshape(NKT, KT, D + 1).transpose(1, 0, 2).reshape(128, -1)
            )
        in_maps.append(
            {
                "kq": kq.astype(ml_dtypes.bfloat16),
                "vaug": va.astype(ml_dtypes.bfloat16),
            }
        )
    return in_maps


def _unshard(res, core_units):
    """Host finish: normalize by the denominator row and transpose."""
    out = np.empty((B, LQ, D), np.float32)
    for c in range(N_CORES):
        o = res.results[c]["o"]  # [NSLOT, 65, 512]
        for s, (b, qt) in enumerate(core_units[c]):
            ot = o[s]
            out[b, qt * QT : (qt + 1) * QT, :] = (ot[:D] / ot[D : D + 1]).T
    return out


def kernel(queries, keys, values, valid_lens):
    core_units, nv_slots = _plan(valid_lens)
    nc = _build_module(nv_slots)
    in_maps = _shard_inputs(queries, keys, values, valid_lens, core_units)
    res = bass_utils.run_bass_kernel_spmd(nc, in_maps, core_ids=list(range(N_CORES)))
    return _unshard(res, core_units)


# revision 8
# speedup vs baseline: 1.2310x; 1.0116x over previous
"""Fused masked-softmax attention (DotProductAttention) for 8 TRN2 NeuronCores.

Problem: B=16 batches of Q[2048,64] @ K[2048,64]^T -> mask cols >= valid_len
to -1e6 -> softmax -> @ V[2048,64].

Work decomposition: each batch splits into 4 q-quarters of 512 rows -> 64
units.  Units are sorted by valid k-tile count nv = ceil(valid_len/128) and
dealt into 8 SPMD slots of 8 units (one per core); the compiled program
runs slot s with a static nv_s = max over that slot's units.  K-tiles
wholly past a unit's valid_len contribute exactly 0 (the mask row drives
exp to underflow), so extra tiles are harmless and skipped tiles exact.

v4 design ("ACT is the roofline"): the scalar engine's exp throughput
(1 elem/cycle/lane @1.2GHz = 427ns per 128x512 k-tile) is the hard floor
(~28.6us for the ~67 k-tiles/core this input needs).  Everything else is
arranged to keep ACT saturated from ~9us to the end:

  * mm1:  S^T chunk [128k, 512q] = kTa.T @ qTa with AUGMENTED bf16
    operands kTa=[K^T; mask_row], qTa=[Q^T; ones] (65-deep contraction).
  * exp:  ACT engine, exp(0.125*x), PSUM -> one big persistent SBUF tile
    (exps_all).  Score groups are GLOBAL (flat across slot boundaries):
    uniform 3-wide merged activations minimize the ~150ns/instr bubble.
  * mm2:  O^T_aug [65, 512q] = sum_k Vaug[kt].T @ expS^T[kt], Vaug=[V|ones]
    -> row 64 = softmax denominator in fp32 PSUM.
  * finish: DVE copies PSUM->SBUF [65,512], DMA straight out.  The
    division by the denominator AND the transpose back to [q, d] happen
    ON HOST (numpy) - no PE transposes, no reciprocal, no identity matrix,
    no gpsimd anywhere.

Scheduling: HAM duty-cycles the PE (cold 1.2GHz until ~3.4us of sustained
high-intensity work; bf16 65-row matmuls alone do NOT promote).  A short
fp32 warmup burst (DVE-memset tile) runs during the input DMAs, and the
first RUNWAY act-groups are pure mm1 (cold mm1 feeds ACT with margin
1335 < 1640 ns/group).  mm2 is deferred into a queue drained between
groups under a per-group budget: light while possibly cold, heavy once
warm.  Optional fp32 N=128 filler MMs keep PE duty high during the runway
so the MID window doesn't demote the clock before the drain phase.
"""

import functools

import numpy as np
import ml_dtypes

import concourse.bacc as bacc
import concourse.tile as tile
from concourse import mybir
from concourse import bass_utils

B, LQ, LKV, D = 16, 2048, 2048, 64
N_CORES = 8
KT = 128            # k-tile (partition dim of S^T)
QT = 512            # q-rows per unit (= PSUM bank free dim)
NKT = LKV // KT     # 16
NSLOT = (B * LQ) // (N_CORES * QT)  # 8 units per core
GROUP = 3           # k-tiles per PSUM score tile / merged activation
MASK_RAW = -8.0e6   # * 0.125 scale == -1e6 (reference MASK_VALUE)
F32 = mybir.dt.float32
BF16 = mybir.dt.bfloat16

# --- schedule knobs ---
WARM_MM = 5         # fp32 warmup matmuls before the first mm1
DRAIN_START = 5     # first act-group index that drains deferred mm2
FILL_MM = 4         # fp32 filler MMs per pre-drain group (HAM duty hold);
                    # emitted AFTER the act so its count-semaphore wait
                    # does not include them
MARGIN = 2          # mm2 item needs exps emitted >= MARGIN groups back


def _widths(nv):
    """Split nv k-tiles into mm2 burst groups of width <=3."""
    threes, rem = divmod(nv, 3)
    out = [3] * threes
    if rem:
        out.append(rem)
    return out


@functools.lru_cache(maxsize=4)
def _build_module(nv_slots):
    nc = bacc.Bacc(None)
    # kq slab per slot: [qta (512 cols) | kta (nv*128 cols)] -> one DMA each
    kq_d = nc.dram_tensor("kq", [NSLOT, D + 1, QT + LKV], BF16, kind="ExternalInput")
    vau_d = nc.dram_tensor("vaug", [128, NSLOT * NKT * (D + 1)], BF16, kind="ExternalInput")
    out_d = nc.dram_tensor("o", [NSLOT, D + 1, QT], F32, kind="ExternalOutput")

    ntile = sum(nv_slots)
    # global flat tile list: (slot, n) in consumption order
    tiles = [(s, n) for s in range(NSLOT) for n in range(nv_slots[s])]
    base = [sum(nv_slots[:s]) for s in range(NSLOT)]
    # global act groups: first group narrow (earliest possible ACT start),
    # then uniform 3-wide
    gwidths = []
    rem = ntile
    first = 2 if ntile % 3 == 2 else (1 if ntile % 3 == 1 else 3)
    gwidths.append(first)
    rem -= first
    while rem:
        w = min(3, rem)
        gwidths.append(w)
        rem -= w
    ngrp = len(gwidths)

    with tile.TileContext(nc) as tc:
        with (
            tc.tile_pool(name="weights", bufs=1) as wpool,
            tc.tile_pool(name="exps", bufs=1) as epool,
            tc.tile_pool(name="ot", bufs=4) as otpool,
            tc.tile_pool(name="ps_s", bufs=2, space="PSUM") as ps_s,
            tc.tile_pool(name="ps_o", bufs=2, space="PSUM") as ps_o,
        ):
            # Warmup operand: DVE-memset fp32 ones (no DMA, no gpsimd).
            wrm = wpool.tile([128, 128], F32, tag="wrm")
            nc.vector.memset(wrm, 1.0)

            kq_s = [
                wpool.tile(
                    [D + 1, QT + nv_slots[s] * KT], BF16, tag=f"kq{s}", name=f"kq{s}"
                )
                for s in range(NSLOT)
            ]
            qta_s = [kq_s[s][:, :QT] for s in range(NSLOT)]

            def kta(s, n):
                return kq_s[s][:, QT + n * KT : QT + (n + 1) * KT]

            vaug_s = [
                wpool.tile([128, nv_slots[s] * (D + 1)], BF16, tag=f"vaug{s}", name=f"vaug{s}")
                for s in range(NSLOT)
            ]
            exps_all = epool.tile([128, ntile * QT], BF16, tag="exps", name="exps_all")

            # Input DMAs, all on the SP ring in consumption order.  The head
            # chunk (qta0 + first-group kta0) goes first; vaug loads are
            # interleaved after each later slot so they land well before the
            # mm2 drain phase reaches them.
            # slot 0 in three chunks so early act-groups aren't gated on
            # one big transfer
            cuts = [0, QT + gwidths[0] * KT + GROUP * KT]
            cuts.append(min(QT + nv_slots[0] * KT, cuts[1] + 2 * GROUP * KT))
            cuts.append(QT + nv_slots[0] * KT)
            for a, b in zip(cuts, cuts[1:]):
                if b > a:
                    nc.sync.dma_start(out=kq_s[0][:, a:b], in_=kq_d[0, :, a:b])
            for s in range(1, NSLOT):
                nc.sync.dma_start(
                    out=kq_s[s], in_=kq_d[s, :, : QT + nv_slots[s] * KT]
                )
                nc.sync.dma_start(
                    out=vaug_s[s - 1],
                    in_=vau_d[:, (s - 1) * NKT * (D + 1) : ((s - 1) * NKT + nv_slots[s - 1]) * (D + 1)],
                )
            nc.sync.dma_start(
                out=vaug_s[NSLOT - 1],
                in_=vau_d[:, (NSLOT - 1) * NKT * (D + 1) : ((NSLOT - 1) * NKT + nv_slots[NSLOT - 1]) * (D + 1)],
            )

            def emit_warm(n):
                for _ in range(n):
                    wp = ps_o.tile([128, 128], F32, tag="po", name="warm")
                    nc.tensor.matmul(wp, lhsT=wrm, rhs=wrm, start=True, stop=True)

            emit_warm(WARM_MM)

            po_of = {}

            def emit_mm2(s, g, w):
                """Deferred attn@V accumulation for slot s, k-tiles [g, g+w)."""
                nv = nv_slots[s]
                if g == 0:
                    po_of[s] = ps_o.tile([D + 1, QT], F32, tag="po", name="po")
                po = po_of[s]
                for j in range(w):
                    n = g + j
                    t = base[s] + n
                    nc.tensor.matmul(
                        po,
                        lhsT=vaug_s[s][:, n * (D + 1) : (n + 1) * (D + 1)],
                        rhs=exps_all[:, t * QT : (t + 1) * QT],
                        start=(n == 0),
                        stop=(n == nv - 1),
                        skip_group_check=True,
                    )
                if g + w == nv:
                    ot = otpool.tile([D + 1, QT], F32, tag="ot", name="ot")
                    # tail-critical last slot: copy on the (idle) ACT engine
                    # so it doesn't queue behind the DVE's s6 copy
                    if s == NSLOT - 1:
                        nc.scalar.copy(ot, po)
                    else:
                        nc.vector.tensor_copy(ot, po)
                    nc.sync.dma_start(out=out_d[s], in_=ot)

            # mm2 queue: per-slot bursts in <=3-wide chunks, slot order.
            m2q = []
            for s in range(NSLOT):
                g = 0
                for w in _widths(nv_slots[s]):
                    m2q.append((s, g, w))
                    g += w
            qi = 0

            # act coverage (in tiles) after each emitted group
            cover = []
            acc = 0
            for w in gwidths:
                acc += w
                cover.append(acc)

            t0 = 0
            for gi in range(ngrp):
                w = gwidths[gi]
                # drain deferred mm2 under budget; exps must be MARGIN
                # groups back so the FIFO tensor queue never head-blocks.
                # Sustainable drain rate is ~4.6 mm2/group (PE slack vs the
                # 1640ns act pace): light while possibly cold, then 5/4.
                if gi < DRAIN_START:
                    budget = 0
                elif gi < DRAIN_START + 2:
                    budget = 2
                else:
                    budget = 5 if (gi - DRAIN_START) % 2 == 0 else 4
                ready_cover = cover[gi - MARGIN] if gi >= MARGIN else 0
                while budget > 0 and qi < len(m2q):
                    s2, g2, w2 = m2q[qi]
                    if base[s2] + g2 + w2 > ready_cover or w2 > budget:
                        break
                    emit_mm2(s2, g2, w2)
                    budget -= w2
                    qi += 1
                # mm1 group gi -> one 3-bank PSUM tile -> merged exp
                st = ps_s.tile([128, GROUP * QT], F32, tag="st", name="st")
                for j in range(w):
                    s, n = tiles[t0 + j]
                    nc.tensor.matmul(
                        st[:, j * QT : (j + 1) * QT],
                        lhsT=kta(s, n),
                        rhs=qta_s[s],
                        start=True,
                        stop=True,
                    )
                nc.scalar.activation(
                    out=exps_all[:, t0 * QT : (t0 + w) * QT],
                    in_=st[:, : w * QT],
                    func=mybir.ActivationFunctionType.Exp,
                    scale=0.125,
                )
                if 1 <= gi < DRAIN_START and FILL_MM:
                    emit_warm(FILL_MM)
                
                t0 += w
            while qi < len(m2q):
                emit_mm2(*m2q[qi])
                qi += 1

    nc.compile()
    return nc


def _plan(valid_lens):
    """Sort the 64 (batch, q-quarter) units by valid k-tile count and deal
    them into NSLOT slots of one unit per core.  Returns (core_units,
    nv_slots) where core_units[c][s] = (batch, quarter)."""
    VL = np.asarray(valid_lens).astype(np.int64)
    nv = np.maximum(1, np.minimum(NKT, (VL + KT - 1) // KT))
    qpb = LQ // QT  # quarters per batch
    unit_nv = np.repeat(nv, qpb)
    order = np.argsort(-unit_nv, kind="stable")
    core_units = [
        [(int(order[NSLOT * s + c]) // qpb, int(order[NSLOT * s + c]) % qpb) for s in range(NSLOT)]
        for c in range(N_CORES)
    ]
    nv_slots = tuple(int(unit_nv[order[NSLOT * s]]) for s in range(NSLOT))
    return core_units, nv_slots


def _shard_inputs(queries, keys, values, valid_lens, core_units):
    """Host-side layout per core: stacked per-unit augmented operands."""
    Q = np.asarray(queries, dtype=np.float32)
    K = np.asarray(keys, dtype=np.float32)
    V = np.asarray(values, dtype=np.float32)
    VL = np.asarray(valid_lens).astype(np.int64)

    cols = np.arange(LKV, dtype=np.int64)
    ones_row = np.ones((1, QT), np.float32)
    in_maps = []
    for c in range(N_CORES):
        kq = np.zeros((NSLOT, D + 1, QT + LKV), np.float32)
        va = np.empty((128, NSLOT * NKT * (D + 1)), np.float32)
        for s, (b, qt) in enumerate(core_units[c]):
            kq[s, :, :QT] = np.concatenate(
                [Q[b, qt * QT : (qt + 1) * QT, :].T, ones_row], axis=0
            )
            mask = np.where(cols >= VL[b], MASK_RAW, 0.0).astype(np.float32)
            kq[s, :, QT : QT + LKV] = np.concatenate([K[b].T, mask[None, :]], axis=0)
            vb = np.concatenate([V[b], np.ones((LKV, 1), np.float32)], axis=-1)
            va[:, s * NKT * (D + 1) : (s + 1) * NKT * (D + 1)] = (
                vb.reshape(NKT, KT, D + 1).transpose(1, 0, 2).reshape(128, -1)
            )
        in_maps.append(
            {
                "kq": kq.astype(ml_dtypes.bfloat16),
                "vaug": va.astype(ml_dtypes.bfloat16),
            }
        )
    return in_maps


def _unshard(res, core_units):
    """Host finish: normalize by the denominator row and transpose."""
    out = np.empty((B, LQ, D), np.float32)
    for c in range(N_CORES):
        o = res.results[c]["o"]  # [NSLOT, 65, 512]
        for s, (b, qt) in enumerate(core_units[c]):
            ot = o[s]
            out[b, qt * QT : (qt + 1) * QT, :] = (ot[:D] / ot[D : D + 1]).T
    return out


def kernel(queries, keys, values, valid_lens):
    core_units, nv_slots = _plan(valid_lens)
    nc = _build_module(nv_slots)
    in_maps = _shard_inputs(queries, keys, values, valid_lens, core_units)
    res = bass_utils.run_bass_kernel_spmd(nc, in_maps, core_ids=list(range(N_CORES)))
    return _unshard(res, core_units)
